# revision 2
# baseline (speedup 1.0000x reference)
"""GCN layer kernel for 8 trn2 NeuronCores (SPMD, single launch), v3.

Math:  out = D^-1/2 (A+I) D^-1/2 X W^T + b
     = dinv_r * (A @ (dinv_k * U)) + dinv_r^2 * U_local + b,  U = X W^T

Row-shard A (strip = 1024 rows/core).  Per core:
  - U tiles (X@W^T) computed up front on PE from a resident X^T (bf16).
  - Stream the strip in [128,1024] fp32 chunks; ScalarE casts each chunk to
    bf16 with accum_out yielding row-sum partials for free; PE transposes the
    bf16 tiles (1-pass weight load, ~2x faster than fp32 transposes); DVE+Act
    drain the transposed PSUM groups into a resident bf16 A^T strip.
  - Degree exchange via TWO pipelined CC AllGathers: #1 after row-block 3
    (flies while blocks 4-7 stream, latency hidden), #2 after block 7 (only
    its ~tens-of-us latency is exposed).
  - Phase-2 matmuls stream in "waves": once a wave's dinv is known, its
    (row-block x k-block-group) products accumulate into PSUM in contiguous
    8-matmul groups and are drain-added into an SBUF Z accumulator (PSUM
    accumulation groups must not interleave within a bank).  After AllGather
    #1 everything except the last four waves overlaps the streaming.
"""

import numpy as np
import ml_dtypes

N = 8192          # nodes
F = 128           # in/out feature dim
NCORES = 8
SR = N // NCORES  # strip rows per core = 1024
P = 128           # partitions / tile edge
IT = SR // P      # 8 row tiles per strip
JT = N // P       # 64 contraction tiles
CH = 2048         # chunk columns for DMA
NCH = N // CH     # 8 chunks per row-tile
TPC = CH // P     # 8 col-tiles per chunk
HC = CH // 2      # cast half-chunk
HB = IT // 2      # row-blocks covered per AllGather = 4

_CACHE = {}


def _build_nc():
    import concourse.mybir as mybir
    from concourse import bass
    from concourse.tile import TileContext

    f32 = mybir.dt.float32
    bf16 = mybir.dt.bfloat16
    AF = mybir.ActivationFunctionType

    nc = bass.Bass(num_devices=NCORES)

    A_s = nc.declare_dram_parameter("a_strip", [SR, N], f32, False)
    Xt = nc.declare_dram_parameter("xt_bf", [P, N], bf16, False)   # X^T, bf16
    XtL = nc.declare_dram_parameter("xt_loc", [P, SR], bf16, False)
    Wt = nc.declare_dram_parameter("wt", [P, F], f32, False)       # W^T
    Bb = nc.declare_dram_parameter("b_bc", [P, F], f32, False)     # bias bcast
    IdB = nc.declare_dram_parameter("ident_bf", [P, P], bf16, False)
    IdF = nc.declare_dram_parameter("ident_f32", [P, P], f32, False)
    out = nc.declare_dram_parameter("out", [SR, F], f32, True)

    degW = nc.dram_tensor("cc_warm_in", [1, P], f32)
    degWA = nc.dram_tensor("cc_warm_out", [NCORES, P], f32, addr_space="Shared")
    degL = [nc.dram_tensor(f"deg_l{h}", [HB, P], f32) for h in range(2)]
    degA = [nc.dram_tensor(f"deg_a{h}", [NCORES * HB, P], f32,
                           addr_space="Shared") for h in range(2)]

    with TileContext(nc) as tc:
        with tc.tile_pool(name="const", bufs=1) as constp, \
             tc.tile_pool(name="big", bufs=1) as bigp, \
             tc.tile_pool(name="chf", bufs=3) as chfp, \
             tc.tile_pool(name="chb", bufs=2) as chbp, \
             tc.tile_pool(name="small", bufs=1) as smallp, \
             tc.tile_pool(name="outs", bufs=2) as outp, \
             tc.tile_pool(name="trps", bufs=3, space="PSUM") as trps, \
             tc.tile_pool(name="accs", bufs=3, space="PSUM") as accs, \
             tc.tile_pool(name="ups", bufs=2, space="PSUM") as ups:

            # ---- constants ----
            ident = constp.tile([P, P], bf16)
            nc.sync.dma_start(out=ident[:, :], in_=IdB[:, :])
            identF = constp.tile([P, P], f32)
            nc.sync.dma_start(out=identF[:, :], in_=IdF[:, :])
            wt_sb = constp.tile([P, F], f32)
            nc.sync.dma_start(out=wt_sb[:, :], in_=Wt[:, :])
            bb_sb = constp.tile([P, F], f32)
            nc.sync.dma_start(out=bb_sb[:, :], in_=Bb[:, :])
            wt_bf = constp.tile([P, F], bf16)
            nc.vector.tensor_copy(wt_bf[:, :], wt_sb[:, :])

            # ---- persistent big buffers ----
            xt_sb = bigp.tile([P, N], bf16)
            nc.gpsimd.dma_start(out=xt_sb[:, :], in_=Xt[:, :])
            xtl_sb = constp.tile([P, SR], bf16)
            nc.gpsimd.dma_start(out=xtl_sb[:, :], in_=XtL[:, :])
            At = bigp.tile([P, IT * JT * P], bf16)   # transposed strip (bf16)
            Yp = bigp.tile([P, JT * F], bf16)        # U then Y (scaled) tiles
            Yloc = bigp.tile([P, IT * F], bf16)      # local U
            Zacc = bigp.tile([P, IT * F], f32)       # Z accumulator (SBUF)
            rsp = smallp.tile([P, IT * 2 * NCH], f32)  # row-sum partials
            rs = smallp.tile([P, IT], f32)           # per-block local row sums
            dinvL = smallp.tile([P, IT], f32)        # local dinv (rows)
            # gathered-degree dinv, transposed: half h col 4c+m = global
            # block 8c + h*4 + m
            dinvT = [smallp.tile([P, NCORES * HB], f32, name=f"dinvT{h}")
                     for h in range(2)]

            # prewarm the CC cores so the first real AllGather is fast
            wz = smallp.tile([1, P], f32)
            nc.vector.memset(wz[:, :], 0.0)
            nc.gpsimd.dma_start(out=degW[:, :], in_=wz[:, :])
            nc.gpsimd.collective_compute(
                "AllGather", mybir.AluOpType.bypass,
                replica_groups=[list(range(NCORES))],
                ins=[degW[:, :]], outs=[degWA[:, :]],
            )

            # ---- U = X @ W^T (PE, early; fills Yp unscaled) ----
            for jt in range(JT):
                ut = ups.tile([P, F], f32, tag="u")
                nc.tensor.matmul(
                    ut[:, :], xt_sb[:, jt * P:(jt + 1) * P], wt_bf[:, :],
                    start=True, stop=True,
                )
                nc.vector.tensor_copy(Yp[:, jt * F:(jt + 1) * F], ut[:, :])
            for m in range(IT):
                ut = ups.tile([P, F], f32, tag="u")
                nc.tensor.matmul(
                    ut[:, :], xtl_sb[:, m * P:(m + 1) * P], wt_bf[:, :],
                    start=True, stop=True,
                )
                nc.vector.tensor_copy(Yloc[:, m * F:(m + 1) * F], ut[:, :])

            zfirst = [True] * IT

            def wave_group(row, waves):
                # one contiguous PSUM accumulation group over len(waves)*8
                # matmuls, then a single drain-add into Zacc
                pp = accs.tile([P, F], f32, tag="z")
                jts = [8 * i + w for w in waves for i in range(NCORES)]
                for n, jt in enumerate(jts):
                    nc.tensor.matmul(
                        pp[:, :],
                        At[:, (row * JT + jt) * P:(row * JT + jt + 1) * P],
                        Yp[:, jt * F:(jt + 1) * F],
                        start=(n == 0), stop=(n == len(jts) - 1),
                    )
                dst = Zacc[:, row * F:(row + 1) * F]
                if zfirst[row]:
                    nc.vector.tensor_copy(dst, pp[:, :])
                    zfirst[row] = False
                else:
                    nc.vector.tensor_add(dst, dst, pp[:, :])

            def rsqrt_newton(dst, src, shape):
                # dst = (src+1)^-1/2 with one Newton step (sqrt LUT fixup)
                sq = smallp.tile(shape, f32)
                nc.scalar.activation(sq, src, AF.Sqrt, bias=1.0)
                r0 = smallp.tile(shape, f32)
                nc.vector.reciprocal(r0, sq)
                d1 = smallp.tile(shape, f32)
                nc.vector.tensor_scalar_add(d1, src, 1.0)
                t = smallp.tile(shape, f32)
                nc.vector.tensor_mul(t, r0, r0)
                nc.vector.tensor_mul(t, t, d1)
                nc.scalar.activation(t, t, AF.Copy, bias=1.5, scale=-0.5)
                nc.vector.tensor_mul(dst, r0, t)

            dgs = []

            def gather_launch(h):
                # transpose local rowsums [P, HB] -> [HB, P] (tiny PE op, no
                # stall), then DMA + AllGather + result DMA all issued from
                # the otherwise-idle Pool queue so SP keeps streaming chunks
                ps = ups.tile([HB, P], f32, tag="u")
                nc.tensor.transpose(ps[:, :], rs[:, h * HB:(h + 1) * HB],
                                    identF[:, :])
                rsT = smallp.tile([HB, P], f32, name=f"rsT{h}")
                nc.vector.tensor_copy(rsT[:, :], ps[:, :])
                nc.gpsimd.dma_start(out=degL[h][:, :], in_=rsT[:, :])
                nc.gpsimd.collective_compute(
                    "AllGather", mybir.AluOpType.bypass,
                    replica_groups=[list(range(NCORES))],
                    ins=[degL[h][:, :]], outs=[degA[h][:, :]],
                )
                dg = smallp.tile([NCORES * HB, P], f32, name=f"dg{h}")
                nc.gpsimd.dma_start(out=dg[:, :], in_=degA[h][:, :])
                dgs.append(dg)

            def gather_finish(h):
                # post-stream: rsqrt the gathered degree, transpose into
                # per-partition scalar layout
                di = smallp.tile([NCORES * HB, P], f32, name=f"di{h}")
                rsqrt_newton(di[:, :], dgs[h][:, :], [NCORES * HB, P])
                ps2 = ups.tile([P, NCORES * HB], f32, tag="u")
                nc.tensor.transpose(ps2[:, :], di[:, :],
                                    identF[0:NCORES * HB, 0:NCORES * HB])
                nc.vector.tensor_copy(dinvT[h][:, :], ps2[:, :])

            def scale_wave(wave):
                # scale the 8 Y tiles of this wave (ScalarE; DVE is busy with
                # drains); Yloc scaled via local dinv
                h, m = wave // HB, wave % HB
                nc.vector.tensor_scalar_mul(
                    Yloc[:, wave * F:(wave + 1) * F],
                    Yloc[:, wave * F:(wave + 1) * F],
                    dinvL[:, wave:wave + 1],
                )
                for c in range(NCORES):
                    jt = 8 * c + wave
                    col = HB * c + m
                    nc.scalar.activation(
                        Yp[:, jt * F:(jt + 1) * F], Yp[:, jt * F:(jt + 1) * F],
                        AF.Copy, scale=dinvT[h][:, col:col + 1],
                    )

            # ---- stream strip; cast+rowsum, transpose, waves ----
            ndr = 0
            for it in range(IT):
                for g in range(NCH):
                    chf = chfp.tile([P, CH], f32)
                    nc.sync.dma_start(
                        out=chf[:, :],
                        in_=A_s[it * P:(it + 1) * P, g * CH:(g + 1) * CH],
                    )
                    for hf in range(2):
                        chb = chbp.tile([P, HC], bf16)
                        nc.scalar.activation(
                            chb[:, :], chf[:, hf * HC:(hf + 1) * HC], AF.Copy,
                            accum_out=rsp[:, it * 2 * NCH + 2 * g + hf:
                                          it * 2 * NCH + 2 * g + hf + 1],
                        )
                        ps = trps.tile([P, HC], bf16)
                        for q in range(HC // P):
                            sub = chb[:, q * P:(q + 1) * P]
                            nc.tensor.transpose(
                                ps[:, q * P:(q + 1) * P], sub, ident[:, :],
                            )
                        jt0 = g * (CH // P) + hf * (HC // P)
                        dest = At[:, (it * JT + jt0) * P:
                                  (it * JT + jt0 + HC // P) * P]
                        nc.vector.tensor_copy(dest, ps[:, :])
                        ndr += 1

                # combine row-sum partials for this block
                nc.vector.tensor_reduce(
                    rs[:, it:it + 1], rsp[:, it * 2 * NCH:(it + 1) * 2 * NCH],
                    axis=mybir.AxisListType.X, op=mybir.AluOpType.add,
                )
                if it == HB - 1 or it == IT - 1:
                    # fire the AllGather from the Pool queue; it flies while
                    # streaming continues
                    gather_launch(it // HB)

            # ---- post-stream: dinv, scales, wave matmuls ----
            rsqrt_newton(dinvL[:, :], rs[:, :], [P, IT])
            gather_finish(0)
            for w in range(HB):
                scale_wave(w)
            for row in range(IT):
                wave_group(row, range(HB))
            gather_finish(1)
            for w in range(HB, IT):
                scale_wave(w)
            for row in range(IT):
                wave_group(row, range(HB, IT))

            # ---- epilogue ----
            for row in range(IT):
                t1 = outp.tile([P, F], f32)
                nc.vector.tensor_add(
                    t1[:, :], Zacc[:, row * F:(row + 1) * F],
                    Yloc[:, row * F:(row + 1) * F],
                )
                nc.vector.tensor_scalar_mul(t1[:, :], t1[:, :],
                                            dinvL[:, row:row + 1])
                nc.vector.tensor_add(t1[:, :], t1[:, :], bb_sb[:, :])
                nc.sync.dma_start(out=out[row * P:(row + 1) * P, :], in_=t1[:, :])

    return nc


_NO_SPLIT_TYPES = ("InstEventSemaphore", "InstSemaphore", "InstTrigger")


def _split_drain_waits(nc, max_waits=1):
    """This walrus build only encodes one sem-wait per instruction; hoist
    extras onto preceding same-engine NOPs (monotonic sems => equivalent)."""
    import concourse.mybir as mybir
    for fn in nc.m.functions:
        for blk in fn.blocks:
            newlist = []
            for ins in blk.instructions:
                si = getattr(ins, "sync_info", None)
                tname = type(ins).__name__
                if si is not None and si.on_wait and len(si.on_wait) > max_waits \
                        and not any(tname.startswith(t) for t in _NO_SPLIT_TYPES):
                    waits = list(si.on_wait)
                    for j, w in enumerate(waits[max_waits:]):
                        newlist.append(mybir.InstNoOp(
                            name=f"{ins.name}-w{j}", engine=ins.engine,
                            ins=[], outs=[],
                            sync_info=mybir.SyncInfo(on_wait=[w], on_update=[]),
                        ))
                    si.on_wait = waits[:max_waits]
                newlist.append(ins)
            blk.instructions[:] = newlist


def _get_nc():
    if "nc" not in _CACHE:
        nc = _build_nc()
        _split_drain_waits(nc)
        _CACHE["nc"] = nc
    return _CACHE["nc"]


def _make_in_maps(X, A, W, b):
    bf16 = ml_dtypes.bfloat16
    X = np.ascontiguousarray(np.asarray(X, dtype=np.float32))
    A = np.ascontiguousarray(np.asarray(A, dtype=np.float32))
    W = np.ascontiguousarray(np.asarray(W, dtype=np.float32))
    b = np.ascontiguousarray(np.asarray(b, dtype=np.float32))
    Xt_bf = np.ascontiguousarray(X.T).astype(bf16)
    Wt = np.ascontiguousarray(W.T)
    Bb = np.ascontiguousarray(np.tile(b[None, :], (P, 1)))
    IdB = np.eye(P, dtype=np.float32).astype(bf16)
    IdF = np.eye(P, dtype=np.float32)
    in_maps = []
    for c in range(NCORES):
        in_maps.append({
            "a_strip": np.ascontiguousarray(A[c * SR:(c + 1) * SR, :]),
            "xt_bf": Xt_bf,
            "xt_loc": np.ascontiguousarray(Xt_bf[:, c * SR:(c + 1) * SR]),
            "wt": Wt,
            "b_bc": Bb,
            "ident_bf": IdB,
            "ident_f32": IdF,
        })
    return in_maps


def _install_ntff_hook():
    """This image's antenv lacks axon_hooks; synthesize it so trace=True
    can reach the terminal's NTFF capture via the libaxon ctypes hook."""
    import sys
    import types
    if "antenv.axon_hooks" in sys.modules:
        return
    try:
        from trn_agent_boot.trn_boot import _ntff_profile_via_ctypes
        hook = _ntff_profile_via_ctypes("/opt/axon/libaxon_pjrt.so")
    except Exception:
        hook = None
    mod = types.ModuleType("antenv.axon_hooks")
    mod._hook = hook
    mod.get_axon_ntff_profile_hook = lambda: mod._hook
    def _set(h):
        mod._hook = h
    mod.set_axon_ntff_profile_hook = _set
    sys.modules["antenv.axon_hooks"] = mod
    import antenv
    antenv.axon_hooks = mod
    # the artifact upload needs a bucket this sandbox doesn't have
    import concourse.bass_utils as bu
    bu.upload_artifacts = lambda tmpdir: f"local:{tmpdir}"


def run(X, A, W, b, trace=False, **trace_kwargs):
    """Run on hardware; returns (output, BassKernelResults)."""
    from concourse.bass_utils import run_bass_kernel_spmd
    if trace:
        _install_ntff_hook()
    nc = _get_nc()
    in_maps = _make_in_maps(X, A, W, b)
    res = run_bass_kernel_spmd(nc, in_maps, list(range(NCORES)),
                               trace=trace, **trace_kwargs)
    outs = [np.asarray(res.results[c]["out"], dtype=np.float32)
            for c in range(NCORES)]
    return np.concatenate(outs, axis=0), res


def kernel(X, A, W, b):
    out, _ = run(X, A, W, b, trace=False)
    return out
